# revision 2
# baseline (speedup 1.0000x reference)
"""Causal multi-head attention (B=32, T=512, D=1024, H=16) on 8 Trainium2
NeuronCores, data-parallel over the batch dimension (4 batches per core).

v3 strategy (bf16 operands, fully-resident weights, 2-stage pipelined attention):
  - slot t emits: S matmuls + exp/mask for group g_t, PV matmuls for group
    g_{t-1} (probabilities fully ready -> PV never stalls), and Q/K
    projections for group g_{t+1} interleaved to space out the S tiles.
  - host casts x^T and all four transposed weight matrices to bf16; every
    matmul operand is bf16 (PSUM accumulates f32), so matmuls run at
    1 cycle/row at any moving-dim size and DMA bytes halve.
  - all weights AND all four batches' x^T are loaded to SBUF once at start;
    no weight re-streaming per batch.
  - exact causal spans: k-tile i covers q in [i*128, T) with no padding
    (bf16 needs no >=256 moving dim), so S and PV shrink to 1280 rows/head.
  - schedule weaves the NEXT group's Q/K projections between the current
    group's S/PV matmuls so exp()/mask latency never stalls the PE; V-proj
    of batch b+1 and O-proj of batch b fill the batch boundary.
  - engine assignment: exp + Q-evac + V-evac on Act; K-evac + reciprocal +
    O-normalize + bias-add on DVE; diagonal causal mask-mul + denominator
    broadcast on Pool (gpsimd); PE does only matmuls.
  - softmax denominator via the augmented-V ones column (PV also reduces
    exp(S)); reciprocal reads the PSUM row directly.
"""

import os
import sys

sys.path.insert(0, "/opt/trn_rl_repo")

import numpy as np

import concourse.bass as bass
import concourse.mybir as mybir
import concourse.tile as tile
from concourse import bacc

B, T, D, H = 32, 512, 1024, 16
DK = D // H  # 64
NCORES = 8
BL = B // NCORES  # 4 batches per core
P = 128
CH = D // P  # 8 contraction chunks
TT = T // P  # 4 token tiles
SPAN = 512
NSPANS = D // SPAN
SCALE = 1.0 / float(np.sqrt(DK))

F32 = mybir.dt.float32
MDT = mybir.dt.bfloat16


def _pbcast(ap, parts):
    """View a [1, N] (or [N]) AP as [parts, N] by repeating partition 0."""
    pairs = [list(pair) for pair in ap.ap]
    if len(pairs) >= 2 and pairs[0][1] == 1:
        pairs = pairs[1:]
    return bass.AP(tensor=ap.tensor, offset=ap.offset, ap=[[0, parts]] + pairs)


def build(n_batches=BL, finalize=True):
    nc = bacc.Bacc(None)
    xt = nc.dram_tensor("xt", [n_batches, D, T], MDT, kind="ExternalInput")
    wqt = nc.dram_tensor("wqt", [CH, P, CH, P], MDT, kind="ExternalInput")
    wkt = nc.dram_tensor("wkt", [CH, P, CH, P], MDT, kind="ExternalInput")
    wvt = nc.dram_tensor("wvt", [D, D], MDT, kind="ExternalInput")
    wot = nc.dram_tensor("wot", [D, D], MDT, kind="ExternalInput")
    bo = nc.dram_tensor("bo", [D], F32, kind="ExternalInput")
    msk = nc.dram_tensor("mask", [P, P], MDT, kind="ExternalInput")
    ones = nc.dram_tensor("ones", [P, DK], MDT, kind="ExternalInput")
    y = nc.dram_tensor("y", [n_batches, T, D], F32, kind="ExternalOutput")

    exp = mybir.ActivationFunctionType.Exp
    cpy = mybir.ActivationFunctionType.Copy

    with tile.TileContext(nc) as tc:
        with (
            tc.tile_pool(name="const", bufs=1) as const,
            tc.tile_pool(name="qk", bufs=3) as qk,
            tc.tile_pool(name="ptp", bufs=16) as ptp,
            tc.tile_pool(name="sm", bufs=2) as sm,
            tc.tile_pool(name="yp", bufs=3) as yp,
            tc.tile_pool(name="ps_proj", bufs=3, space="PSUM") as ps_proj,
            tc.tile_pool(name="ps_s", bufs=2, space="PSUM") as ps_s,
            tc.tile_pool(name="ps_o", bufs=3, space="PSUM") as ps_o,
        ):
            # ---- startup DMAs, first-needed first ----
            wq_sb = const.tile([P, CH, CH, P], MDT)
            wk_sb = const.tile([P, CH, CH, P], MDT)
            nc.sync.dma_start(out=wq_sb[:, 0], in_=wqt[0])
            xT_all = const.tile([P, n_batches, CH, T], MDT)
            # chunked so the first Q-proj matmul starts after ~384 KB of DMA
            for c in range(CH):
                nc.sync.dma_start(
                    out=xT_all[:, 0, c, :], in_=xt[0, c * P : (c + 1) * P, :]
                )
            nc.sync.dma_start(out=wk_sb[:, 0], in_=wkt[0])
            wv_sb = const.tile([P, CH, D], MDT)
            nc.sync.dma_start(
                out=wv_sb[:], in_=wvt[:].rearrange("(c p) o -> p c o", p=P)
            )
            mask_sb = const.tile([P, P], MDT)
            nc.sync.dma_start(out=mask_sb[:], in_=msk[:])
            for g in range(1, CH):
                nc.sync.dma_start(out=wq_sb[:, g], in_=wqt[g])
                nc.sync.dma_start(out=wk_sb[:, g], in_=wkt[g])
            wo_sb = const.tile([P, CH, D], MDT)
            nc.sync.dma_start(
                out=wo_sb[:], in_=wot[:].rearrange("(c p) o -> p c o", p=P)
            )
            for b in range(1, n_batches):
                nc.sync.dma_start(
                    out=xT_all[:, b], in_=xt[b].rearrange("(c p) t -> p c t", p=P)
                )
            bias_sb = const.tile([P, D], F32)
            nc.sync.dma_start(out=bias_sb[:], in_=_pbcast(bo[:], P))

            # V layout [t-in-tile, tt, head, dk | 1.0]; double-buffered over
            # batches, ones column written once per buffer.
            vaug0 = const.tile([P, TT, H, DK + 1], MDT)
            vaug1 = const.tile([P, TT, H, DK + 1], MDT)
            vaug_bufs = [vaug0, vaug1]
            for vb in vaug_bufs:
                for tt in range(TT):
                    nc.sync.dma_start(out=vb[:, tt, :, DK], in_=ones[:, 0:H])

            oT0 = const.tile([P, CH, T], MDT)
            oT1 = const.tile([P, CH, T], MDT)
            oT_bufs = [oT0, oT1]

            # ---- emission helpers ----
            def emit_q_proj(b, g):
                qT = qk.tile([P, T], MDT, tag="qT")
                qps = ps_proj.tile([P, T], F32, tag="proj")
                for c in range(CH):
                    nc.tensor.matmul(
                        qps[:],
                        lhsT=wq_sb[:, g, c, :],
                        rhs=xT_all[:, b, c, :],
                        start=(c == 0),
                        stop=(c == CH - 1),
                    )
                with nc.allow_low_precision(reason="bf16 operand staging"):
                    nc.scalar.activation(out=qT[:], in_=qps[:], func=cpy)
                return qT

            def emit_k_proj(b, g):
                kT = qk.tile([P, T], MDT, tag="kT")
                kps = ps_proj.tile([P, T], F32, tag="proj")
                for c in range(CH):
                    nc.tensor.matmul(
                        kps[:],
                        lhsT=wk_sb[:, g, c, :],
                        rhs=xT_all[:, b, c, :],
                        start=(c == 0),
                        stop=(c == CH - 1),
                    )
                with nc.allow_low_precision(reason="bf16 operand staging"):
                    nc.vector.tensor_copy(out=kT[:], in_=kps[:])
                return kT

            def emit_v_proj(b):
                vaug = vaug_bufs[b % 2]
                for s in range(NSPANS):
                    for tt in range(TT):
                        vps = ps_proj.tile([P, SPAN], F32, tag="proj")
                        for c in range(CH):
                            nc.tensor.matmul(
                                vps[:],
                                lhsT=xT_all[:, b, c, tt * P : (tt + 1) * P],
                                rhs=wv_sb[:, c, s * SPAN : (s + 1) * SPAN],
                                start=(c == 0),
                                stop=(c == CH - 1),
                            )
                        hps = SPAN // DK  # heads per span
                        with nc.allow_low_precision(reason="bf16 operand staging"):
                            nc.scalar.activation(
                                out=vaug[:, tt, s * hps : (s + 1) * hps, 0:DK],
                                in_=vps[:].rearrange("p (h d) -> p h d", d=DK),
                                func=cpy,
                            )

            def emit_o_proj(b, stagger=False, corder=tuple(range(CH))):
                oT = oT_bufs[b % 2]

                def mm(yps, tt, s, lo, hi):
                    for ci in range(lo, hi):
                        c = corder[ci]
                        nc.tensor.matmul(
                            yps[:],
                            lhsT=oT[:, c, tt * P : (tt + 1) * P],
                            rhs=wo_sb[:, c, s * SPAN : (s + 1) * SPAN],
                            start=(ci == 0),
                            stop=(ci == CH - 1),
                            skip_group_check=stagger,
                        )

                def evac(yps, tt, s):
                    y_sb = yp.tile([P, SPAN], F32, tag="ysb", name="y_sb")
                    nc.vector.tensor_add(
                        out=y_sb[:],
                        in0=yps[:],
                        in1=bias_sb[:, s * SPAN : (s + 1) * SPAN],
                    )
                    nc.sync.dma_start(
                        out=y[b, tt * P : (tt + 1) * P, s * SPAN : (s + 1) * SPAN],
                        in_=y_sb[:],
                    )

                groups = [(tt, s) for tt in range(TT) for s in range(NSPANS)]
                if stagger:
                    # first two groups: run chunks 0..6 of both before either
                    # c=7, so the final head-group's normalize has time to land
                    ypsA = ps_proj.tile([P, SPAN], F32, tag="proj", name="ypsA")
                    ypsB = ps_proj.tile([P, SPAN], F32, tag="proj", name="ypsB")
                    mm(ypsA, 0, 0, 0, CH - 1)
                    mm(ypsB, 0, 1, 0, CH - 1)
                    mm(ypsA, 0, 0, CH - 1, CH)
                    evac(ypsA, 0, 0)
                    mm(ypsB, 0, 1, CH - 1, CH)
                    evac(ypsB, 0, 1)
                    groups = groups[2:]
                for tt, s in groups:
                    yps = ps_proj.tile([P, SPAN], F32, tag="proj", name="yps")
                    mm(yps, tt, s, 0, CH)
                    evac(yps, tt, s)

            def emit_S(g, qT, kT, hh, i):
                """S matmul + exp + causal mask for head hh, k-tile i."""
                po = hh * DK
                q0 = i * P
                n = T - q0
                sps = ps_s.tile([P, n], F32, tag="sps")
                nc.tensor.matmul(
                    sps[:],
                    lhsT=kT[po : po + DK, i * P : (i + 1) * P],
                    rhs=qT[po : po + DK, q0:T],
                    start=True,
                    stop=True,
                )
                pt = ptp.tile([P, n], MDT, tag="pt")
                with nc.allow_low_precision(reason="bf16 probabilities"):
                    nc.scalar.activation(out=pt[:], in_=sps[:], func=exp, scale=SCALE)
                # zero the strict upper triangle of the diagonal block
                nc.gpsimd.tensor_mul(out=pt[:, 0:P], in0=pt[:, 0:P], in1=mask_sb[:])
                return pt

            def emit_PV(b, g, pts, ops_t, hh, i):
                h = 2 * g + hh
                q0 = i * P
                if i == 0:
                    ops_t[hh] = ps_o.tile([DK + 1, T], F32, tag="ops", name="ops")
                nc.tensor.matmul(
                    ops_t[hh][:, q0:T],
                    lhsT=vaug_bufs[b % 2][:, i, h, :],
                    rhs=pts[hh][i][:],
                    start=(i == 0),
                    stop=(i == TT - 1),
                    skip_group_check=True,
                )

            def emit_posts(b, g, ops_t):
                oT = oT_bufs[b % 2]
                recs = []
                for hh in range(2):
                    rec = sm.tile([1, T], F32, tag=f"rec{hh}", name="rec")
                    nc.vector.reciprocal(out=rec[:], in_=ops_t[hh][DK : DK + 1, :])
                    recs.append(rec)
                bcs = []
                for hh in range(2):
                    bc = sm.tile([DK, T], F32, tag=f"bc{hh}", name="bc")
                    nc.gpsimd.partition_broadcast(bc[:], recs[hh][0:1, :])
                    bcs.append(bc)
                with nc.allow_low_precision(reason="bf16 attention output"):
                    nc.vector.tensor_mul(
                        out=oT[0:DK, g, :], in0=ops_t[0][0:DK, :], in1=bcs[0][:]
                    )
                    otmp = sm.tile([DK, T], MDT, tag="otmp")
                    nc.vector.tensor_mul(
                        out=otmp[:], in0=ops_t[1][0:DK, :], in1=bcs[1][:]
                    )
                    nc.sync.dma_start(out=oT[DK:P, g, :], in_=otmp[:])

            def emit_qk_chunks(b, g, dst, lo, hi, which):
                """Emit contraction chunks [lo, hi) of the Q or K projection
                for group (b, g) into PSUM tile dst."""
                w_sb = wq_sb if which == "q" else wk_sb
                for c in range(lo, hi):
                    nc.tensor.matmul(
                        dst[:],
                        lhsT=w_sb[:, g, c, :],
                        rhs=xT_all[:, b, c, :],
                        start=(c == 0),
                        stop=(c == CH - 1),
                    )

            # ---- main schedule: 2-stage pipelined slots ----
            # last batch processes group 0 last so its odd-head normalize DMA
            # lands well before the O-projection needs chunk 0
            last_gs = list(range(1, CH)) + [0]
            seq = []
            for b in range(n_batches):
                gs = list(range(CH)) if b + 1 < n_batches else last_gs
                seq += [(b, g) for g in gs]
            nslot = len(seq)
            qkt = {seq[0]: (emit_q_proj(*seq[0]), emit_k_proj(*seq[0]))}
            emit_v_proj(0)
            pts_store = {}
            ops_store = {}
            for t in range(nslot + 1):
                cur = seq[t] if t < nslot else None
                prv = seq[t - 1] if t > 0 else None
                nxt = seq[t + 1] if t + 1 < nslot else None

                if nxt is not None:
                    qps = ps_proj.tile([P, T], F32, tag="proj", name="qps")
                    kps = ps_proj.tile([P, T], F32, tag="proj", name="kps")

                if cur is not None:
                    b, g = cur
                    qT, kT = qkt.pop(cur)
                    pts = [[None] * TT, [None] * TT]
                    pts[0][0] = emit_S(g, qT, kT, 0, 0)
                    pts[1][0] = emit_S(g, qT, kT, 1, 0)
                    if nxt is not None:
                        emit_qk_chunks(nxt[0], nxt[1], qps, 0, 6, "q")
                    pts[0][1] = emit_S(g, qT, kT, 0, 1)
                    pts[1][1] = emit_S(g, qT, kT, 1, 1)
                    if nxt is not None:
                        emit_qk_chunks(nxt[0], nxt[1], qps, 6, CH, "q")
                        qT_n = qk.tile([P, T], MDT, tag="qT", name="qT")
                        with nc.allow_low_precision(reason="bf16 operand staging"):
                            nc.scalar.activation(out=qT_n[:], in_=qps[:], func=cpy)
                        emit_qk_chunks(nxt[0], nxt[1], kps, 0, 4, "k")
                    pts[0][2] = emit_S(g, qT, kT, 0, 2)
                    pts[1][2] = emit_S(g, qT, kT, 1, 2)
                    if nxt is not None:
                        emit_qk_chunks(nxt[0], nxt[1], kps, 4, CH, "k")
                        kT_n = qk.tile([P, T], MDT, tag="kT", name="kT")
                        with nc.allow_low_precision(reason="bf16 operand staging"):
                            nc.vector.tensor_copy(out=kT_n[:], in_=kps[:])
                        qkt[nxt] = (qT_n, kT_n)
                    pts[0][3] = emit_S(g, qT, kT, 0, 3)
                    pts_store[cur] = pts

                if prv is not None:
                    pb, pg = prv
                    ppts = pts_store.pop(prv)
                    ops_t = [None, None]
                    emit_PV(pb, pg, ppts, ops_t, 0, 0)

                if cur is not None:
                    pts[1][3] = emit_S(g, qT, kT, 1, 3)

                if prv is not None:
                    emit_PV(pb, pg, ppts, ops_t, 1, 0)
                    for i in range(1, TT):
                        emit_PV(pb, pg, ppts, ops_t, 0, i)
                        emit_PV(pb, pg, ppts, ops_t, 1, i)
                    emit_posts(pb, pg, ops_t)

                # batch boundaries (by slot position within the batch)
                if cur is not None and t % CH == CH - 1 and b + 1 < n_batches:
                    emit_v_proj(b + 1)
                if cur is not None and t % CH == 1 and b > 0:
                    emit_o_proj(b - 1)
            emit_o_proj(
                n_batches - 1, stagger=True, corder=tuple(last_gs)
            )
    if finalize:
        nc.finalize()
    return nc


def host_inputs(x, w_q, w_k, w_v, w_o, b_o):
    import ml_dtypes

    bf16 = ml_dtypes.bfloat16
    xtf = np.ascontiguousarray(
        np.asarray(x, dtype=np.float32).transpose(0, 2, 1)
    ).astype(bf16)  # [B, D, T]

    # [d, o] -> [g, p, c, j]: d = c*128+p, o = g*128+j
    def swz(w):
        wt = np.asarray(w, np.float32).T.reshape(CH, P, CH, P)
        return np.ascontiguousarray(wt.transpose(2, 1, 0, 3)).astype(bf16)

    wqt = swz(w_q)
    wkt = swz(w_k)
    wvt = np.ascontiguousarray(np.asarray(w_v, np.float32).T).astype(bf16)
    wot = np.ascontiguousarray(np.asarray(w_o, np.float32).T).astype(bf16)
    bo = np.asarray(b_o, np.float32)
    kk = np.arange(P)[:, None]
    qq = np.arange(P)[None, :]
    mask = (kk <= qq).astype(np.float32).astype(bf16)
    ones_np = np.ones((P, DK), np.float32).astype(bf16)
    return {
        "xt": xtf,
        "wqt": wqt,
        "wkt": wkt,
        "wvt": wvt,
        "wot": wot,
        "bo": bo,
        "mask": mask,
        "ones": ones_np,
    }


def make_in_maps(host):
    return [
        {
            "xt": host["xt"][c * BL : (c + 1) * BL],
            "wqt": host["wqt"],
            "wkt": host["wkt"],
            "wvt": host["wvt"],
            "wot": host["wot"],
            "bo": host["bo"],
            "mask": host["mask"],
            "ones": host["ones"],
        }
        for c in range(NCORES)
    ]


LAST_RESULTS = None


def kernel(x, w_q, w_k, w_v, w_o, b_o):
    global LAST_RESULTS
    os.environ["BASS_NEVER_TRACE"] = "1"
    from concourse.bass_utils import run_bass_kernel_spmd

    host = host_inputs(x, w_q, w_k, w_v, w_o, b_o)
    nc = build(BL)
    core_ids = list(range(NCORES))
    res = run_bass_kernel_spmd(nc, make_in_maps(host), core_ids)
    LAST_RESULTS = res
    out = np.concatenate([res.results[c]["y"] for c in core_ids], axis=0)
    return out.astype(np.float32)


# revision 9
# speedup vs baseline: 1.1647x; 1.1647x over previous
"""Causal multi-head attention (B=32, T=512, D=1024, H=16) on 8 Trainium2
NeuronCores, data-parallel over the batch dimension (4 batches per core).

v3 strategy (bf16 operands, fully-resident weights, 2-stage pipelined attention):
  - slot t emits: S matmuls + exp/mask for group g_t, PV matmuls for group
    g_{t-1} (probabilities fully ready -> PV never stalls), and Q/K
    projections for group g_{t+1} interleaved to space out the S tiles.
  - host casts x^T and all four transposed weight matrices to bf16; every
    matmul operand is bf16 (PSUM accumulates f32), so matmuls run at
    1 cycle/row at any moving-dim size and DMA bytes halve.
  - all weights AND all four batches' x^T are loaded to SBUF once at start;
    no weight re-streaming per batch.
  - exact causal spans: k-tile i covers q in [i*128, T) with no padding
    (bf16 needs no >=256 moving dim), so S and PV shrink to 1280 rows/head.
  - schedule weaves the NEXT group's Q/K projections between the current
    group's S/PV matmuls so exp()/mask latency never stalls the PE; V-proj
    of batch b+1 and O-proj of batch b fill the batch boundary.
  - engine assignment: exp + Q-evac + V-evac on Act; K-evac + reciprocal +
    O-normalize + bias-add on DVE; diagonal causal mask-mul + denominator
    broadcast on Pool (gpsimd); PE does only matmuls.
  - softmax denominator via the augmented-V ones column (PV also reduces
    exp(S)); reciprocal reads the PSUM row directly.
"""

import os
import sys

sys.path.insert(0, "/opt/trn_rl_repo")

import numpy as np

import concourse.bass as bass
import concourse.mybir as mybir
import concourse.tile as tile
from concourse import bacc

B, T, D, H = 32, 512, 1024, 16
DK = D // H  # 64
NCORES = 8
BL = B // NCORES  # 4 batches per core
P = 128
CH = D // P  # 8 contraction chunks
TT = T // P  # 4 token tiles
SPAN = 512
NSPANS = D // SPAN
SCALE = 1.0 / float(np.sqrt(DK))

F32 = mybir.dt.float32
MDT = mybir.dt.bfloat16


def _pbcast(ap, parts):
    """View a [1, N] (or [N]) AP as [parts, N] by repeating partition 0."""
    pairs = [list(pair) for pair in ap.ap]
    if len(pairs) >= 2 and pairs[0][1] == 1:
        pairs = pairs[1:]
    return bass.AP(tensor=ap.tensor, offset=ap.offset, ap=[[0, parts]] + pairs)


def build(n_batches=BL, finalize=True):
    nc = bacc.Bacc(None)
    xt = nc.dram_tensor("xt", [n_batches, D, T], MDT, kind="ExternalInput")
    wqt = nc.dram_tensor("wqt", [CH, P, CH, P], MDT, kind="ExternalInput")
    wkt = nc.dram_tensor("wkt", [CH, P, CH, P], MDT, kind="ExternalInput")
    wvt = nc.dram_tensor("wvt", [D, D], MDT, kind="ExternalInput")
    wot = nc.dram_tensor("wot", [D, D], MDT, kind="ExternalInput")
    bo = nc.dram_tensor("bo", [D], F32, kind="ExternalInput")
    msk = nc.dram_tensor("mask", [P, P], MDT, kind="ExternalInput")
    ones = nc.dram_tensor("ones", [P, DK], MDT, kind="ExternalInput")
    y = nc.dram_tensor("y", [n_batches, T, D], F32, kind="ExternalOutput")

    exp = mybir.ActivationFunctionType.Exp
    cpy = mybir.ActivationFunctionType.Copy

    with tile.TileContext(nc) as tc:
        with (
            tc.tile_pool(name="const", bufs=1) as const,
            tc.tile_pool(name="qk", bufs=3) as qk,
            tc.tile_pool(name="ptp", bufs=16) as ptp,
            tc.tile_pool(name="sm", bufs=2) as sm,
            tc.tile_pool(name="yp", bufs=3) as yp,
            tc.tile_pool(name="ps_proj", bufs=3, space="PSUM") as ps_proj,
            tc.tile_pool(name="ps_s", bufs=2, space="PSUM") as ps_s,
            tc.tile_pool(name="ps_o", bufs=3, space="PSUM") as ps_o,
        ):
            # ---- startup DMAs, first-needed first ----
            wq_sb = const.tile([P, CH, CH, P], MDT)
            wk_sb = const.tile([P, CH, CH, P], MDT)
            nc.sync.dma_start(out=wq_sb[:, 0], in_=wqt[0])
            xT_all = const.tile([P, n_batches, CH, T], MDT)
            # chunked so the first Q-proj matmul starts after ~384 KB of DMA
            for c in range(CH):
                nc.sync.dma_start(
                    out=xT_all[:, 0, c, :], in_=xt[0, c * P : (c + 1) * P, :]
                )
            nc.sync.dma_start(out=wk_sb[:, 0], in_=wkt[0])
            wv_sb = const.tile([P, CH, D], MDT)
            # split by output-column half: the s=0 V matmuls only need cols
            # [0, 512), so they can start after half the load
            for s in range(NSPANS):
                nc.sync.dma_start(
                    out=wv_sb[:, :, s * SPAN : (s + 1) * SPAN],
                    in_=wvt[:, s * SPAN : (s + 1) * SPAN].rearrange(
                        "(c p) o -> p c o", p=P
                    ),
                )
            mask_sb = const.tile([P, P], MDT)
            nc.sync.dma_start(out=mask_sb[:], in_=msk[:])
            for g in range(1, CH):
                nc.sync.dma_start(out=wq_sb[:, g], in_=wqt[g])
                nc.sync.dma_start(out=wk_sb[:, g], in_=wkt[g])
            wo_sb = const.tile([P, CH, D], MDT)
            nc.sync.dma_start(
                out=wo_sb[:], in_=wot[:].rearrange("(c p) o -> p c o", p=P)
            )
            for b in range(1, n_batches):
                nc.sync.dma_start(
                    out=xT_all[:, b], in_=xt[b].rearrange("(c p) t -> p c t", p=P)
                )
            bias_sb = const.tile([P, D], F32)
            nc.sync.dma_start(out=bias_sb[:], in_=_pbcast(bo[:], P))

            # V layout [t-in-tile, tt, head, dk | 1.0]; double-buffered over
            # batches, ones column written once per buffer.
            vaug0 = const.tile([P, TT, H, DK + 1], MDT)
            vaug1 = const.tile([P, TT, H, DK + 1], MDT)
            vaug_bufs = [vaug0, vaug1]
            for vb in vaug_bufs:
                for tt in range(TT):
                    nc.sync.dma_start(out=vb[:, tt, :, DK], in_=ones[:, 0:H])

            oT0 = const.tile([P, CH, T], MDT)
            oT1 = const.tile([P, CH, T], MDT)
            oT_bufs = [oT0, oT1]

            # ---- emission helpers ----
            def emit_q_proj(b, g):
                qT = qk.tile([P, T], MDT, tag="qT")
                qps = ps_proj.tile([P, T], F32, tag="proj")
                for c in range(CH):
                    nc.tensor.matmul(
                        qps[:],
                        lhsT=wq_sb[:, g, c, :],
                        rhs=xT_all[:, b, c, :],
                        start=(c == 0),
                        stop=(c == CH - 1),
                    )
                with nc.allow_low_precision(reason="bf16 operand staging"):
                    nc.scalar.activation(out=qT[:], in_=qps[:], func=cpy)
                return qT

            def emit_k_proj(b, g):
                kT = qk.tile([P, T], MDT, tag="kT")
                kps = ps_proj.tile([P, T], F32, tag="proj")
                for c in range(CH):
                    nc.tensor.matmul(
                        kps[:],
                        lhsT=wk_sb[:, g, c, :],
                        rhs=xT_all[:, b, c, :],
                        start=(c == 0),
                        stop=(c == CH - 1),
                    )
                with nc.allow_low_precision(reason="bf16 operand staging"):
                    nc.vector.tensor_copy(out=kT[:], in_=kps[:])
                return kT

            def emit_v_proj(b):
                vaug = vaug_bufs[b % 2]
                for s in range(NSPANS):
                    for tt in range(TT):
                        vps = ps_proj.tile([P, SPAN], F32, tag="proj")
                        for c in range(CH):
                            nc.tensor.matmul(
                                vps[:],
                                lhsT=xT_all[:, b, c, tt * P : (tt + 1) * P],
                                rhs=wv_sb[:, c, s * SPAN : (s + 1) * SPAN],
                                start=(c == 0),
                                stop=(c == CH - 1),
                            )
                        hps = SPAN // DK  # heads per span
                        with nc.allow_low_precision(reason="bf16 operand staging"):
                            nc.scalar.activation(
                                out=vaug[:, tt, s * hps : (s + 1) * hps, 0:DK],
                                in_=vps[:].rearrange("p (h d) -> p h d", d=DK),
                                func=cpy,
                            )

            def emit_o_proj(b, stagger=False, corder=tuple(range(CH))):
                oT = oT_bufs[b % 2]

                def mm(yps, tt, s, lo, hi):
                    for ci in range(lo, hi):
                        c = corder[ci]
                        nc.tensor.matmul(
                            yps[:],
                            lhsT=oT[:, c, tt * P : (tt + 1) * P],
                            rhs=wo_sb[:, c, s * SPAN : (s + 1) * SPAN],
                            start=(ci == 0),
                            stop=(ci == CH - 1),
                            skip_group_check=stagger,
                        )

                def evac(yps, tt, s):
                    y_sb = yp.tile([P, SPAN], F32, tag="ysb", name="y_sb")
                    nc.vector.tensor_add(
                        out=y_sb[:],
                        in0=yps[:],
                        in1=bias_sb[:, s * SPAN : (s + 1) * SPAN],
                    )
                    nc.sync.dma_start(
                        out=y[b, tt * P : (tt + 1) * P, s * SPAN : (s + 1) * SPAN],
                        in_=y_sb[:],
                    )

                groups = [(tt, s) for tt in range(TT) for s in range(NSPANS)]
                if stagger:
                    # first three groups: run chunks 0..6 of all before any
                    # final chunk, so the last head-group's normalize (and its
                    # partition-move DMA) has time to land
                    opens = []
                    for tt, s in groups[:3]:
                        yps = ps_proj.tile([P, SPAN], F32, tag="proj", name="yps")
                        mm(yps, tt, s, 0, CH - 1)
                        opens.append((yps, tt, s))
                    for yps, tt, s in opens:
                        mm(yps, tt, s, CH - 1, CH)
                        evac(yps, tt, s)
                    groups = groups[3:]
                for tt, s in groups:
                    yps = ps_proj.tile([P, SPAN], F32, tag="proj", name="yps")
                    mm(yps, tt, s, 0, CH)
                    evac(yps, tt, s)

            def emit_S(g, qT, kT, hh, i):
                """S matmul + exp + causal mask for head hh, k-tile i."""
                po = hh * DK
                q0 = i * P
                n = T - q0
                sps = ps_s.tile([P, n], F32, tag="sps")
                nc.tensor.matmul(
                    sps[:],
                    lhsT=kT[po : po + DK, i * P : (i + 1) * P],
                    rhs=qT[po : po + DK, q0:T],
                    start=True,
                    stop=True,
                )
                pt = ptp.tile([P, n], MDT, tag="pt")
                with nc.allow_low_precision(reason="bf16 probabilities"):
                    nc.scalar.activation(out=pt[:], in_=sps[:], func=exp, scale=SCALE)
                # zero the strict upper triangle of the diagonal block; split
                # across Pool/DVE so neither queues behind the broadcasts
                eng = nc.gpsimd if hh == 0 else nc.vector
                eng.tensor_mul(out=pt[:, 0:P], in0=pt[:, 0:P], in1=mask_sb[:])
                return pt

            def emit_PV(b, g, pts, ops_t, hh, i):
                h = 2 * g + hh
                q0 = i * P
                if i == 0:
                    ops_t[hh] = ps_o.tile([DK + 1, T], F32, tag="ops", name="ops")
                nc.tensor.matmul(
                    ops_t[hh][:, q0:T],
                    lhsT=vaug_bufs[b % 2][:, i, h, :],
                    rhs=pts[hh][i][:],
                    start=(i == 0),
                    stop=(i == TT - 1),
                    skip_group_check=True,
                )

            def emit_post(b, g, ops_t, hh):
                """Normalize one head. The odd head (hh=1) is emitted first in
                the slot: its SBUF->SBUF partition-move DMA is the longest leg
                of the chain, so it overlaps the even head's PV matmuls."""
                oT = oT_bufs[b % 2]
                ops = ops_t[hh]
                rec = sm.tile([1, T], F32, tag=f"rec{hh}", name="rec")
                nc.vector.reciprocal(out=rec[:], in_=ops[DK : DK + 1, :])
                bc = sm.tile([DK, T], F32, tag=f"bc{hh}", name="bc")
                nc.gpsimd.partition_broadcast(bc[:], rec[0:1, :])
                with nc.allow_low_precision(reason="bf16 attention output"):
                    if hh == 0:
                        nc.vector.tensor_mul(
                            out=oT[0:DK, g, :], in0=ops[0:DK, :], in1=bc[:]
                        )
                    else:
                        otmp = sm.tile([DK, T], MDT, tag="otmp")
                        nc.vector.tensor_mul(
                            out=otmp[:], in0=ops[0:DK, :], in1=bc[:]
                        )
                        nc.sync.dma_start(out=oT[DK:P, g, :], in_=otmp[:])

            def emit_qk_chunks(b, g, dst, lo, hi, which):
                """Emit contraction chunks [lo, hi) of the Q or K projection
                for group (b, g) into PSUM tile dst."""
                w_sb = wq_sb if which == "q" else wk_sb
                for c in range(lo, hi):
                    nc.tensor.matmul(
                        dst[:],
                        lhsT=w_sb[:, g, c, :],
                        rhs=xT_all[:, b, c, :],
                        start=(c == 0),
                        stop=(c == CH - 1),
                    )

            # ---- main schedule: 2-stage pipelined slots ----
            # last batch processes group 0 last so its odd-head normalize DMA
            # lands well before the O-projection needs chunk 0
            last_gs = list(range(1, CH)) + [0]
            seq = []
            for b in range(n_batches):
                gs = list(range(CH)) if b + 1 < n_batches else last_gs
                seq += [(b, g) for g in gs]
            nslot = len(seq)
            qkt = {seq[0]: (emit_q_proj(*seq[0]), emit_k_proj(*seq[0]))}
            pts_store = {}
            ops_store = {}
            for t in range(nslot + 1):
                cur = seq[t] if t < nslot else None
                prv = seq[t - 1] if t > 0 else None
                nxt = seq[t + 1] if t + 1 < nslot else None

                if nxt is not None:
                    qps = ps_proj.tile([P, T], F32, tag="proj", name="qps")
                    kps = ps_proj.tile([P, T], F32, tag="proj", name="kps")

                if cur is not None:
                    b, g = cur
                    qT, kT = qkt.pop(cur)
                    pts = [[None] * TT, [None] * TT]
                    pts[0][0] = emit_S(g, qT, kT, 0, 0)
                    pts[1][0] = emit_S(g, qT, kT, 1, 0)
                    if nxt is not None:
                        emit_qk_chunks(nxt[0], nxt[1], qps, 0, 6, "q")
                    pts[0][1] = emit_S(g, qT, kT, 0, 1)
                    pts[1][1] = emit_S(g, qT, kT, 1, 1)
                    if nxt is not None:
                        emit_qk_chunks(nxt[0], nxt[1], qps, 6, CH, "q")
                        qT_n = qk.tile([P, T], MDT, tag="qT", name="qT")
                        with nc.allow_low_precision(reason="bf16 operand staging"):
                            nc.scalar.activation(out=qT_n[:], in_=qps[:], func=cpy)
                        emit_qk_chunks(nxt[0], nxt[1], kps, 0, 4, "k")
                    pts[0][2] = emit_S(g, qT, kT, 0, 2)
                    pts[1][2] = emit_S(g, qT, kT, 1, 2)
                    if nxt is not None:
                        emit_qk_chunks(nxt[0], nxt[1], kps, 4, CH, "k")
                        kT_n = qk.tile([P, T], MDT, tag="kT", name="kT")
                        with nc.allow_low_precision(reason="bf16 operand staging"):
                            nc.vector.tensor_copy(out=kT_n[:], in_=kps[:])
                        qkt[nxt] = (qT_n, kT_n)
                    pts[0][3] = emit_S(g, qT, kT, 0, 3)
                    pts_store[cur] = pts

                if prv is not None:
                    pb, pg = prv
                    ppts = pts_store.pop(prv)
                    ops_t = [None, None]
                    emit_PV(pb, pg, ppts, ops_t, 1, 0)

                if cur is not None:
                    pts[1][3] = emit_S(g, qT, kT, 1, 3)

                if prv is not None:
                    for i in range(1, TT):
                        emit_PV(pb, pg, ppts, ops_t, 1, i)
                    emit_post(pb, pg, ops_t, 1)
                    for i in range(TT):
                        emit_PV(pb, pg, ppts, ops_t, 0, i)
                    emit_post(pb, pg, ops_t, 0)

                # batch boundaries (by slot position within the batch).
                # V(0) lands after slot 0 so its matmuls start once the split
                # wv load is in; later batches prefetch at the previous
                # batch's last slot.
                if t == 0:
                    emit_v_proj(0)
                if cur is not None and t % CH == CH - 1 and b + 1 < n_batches:
                    emit_v_proj(b + 1)
                if cur is not None and t % CH == 1 and b > 0:
                    emit_o_proj(b - 1)
            emit_o_proj(
                n_batches - 1, stagger=True, corder=tuple(last_gs)
            )
    if finalize:
        nc.finalize()
    return nc


def host_inputs(x, w_q, w_k, w_v, w_o, b_o):
    import ml_dtypes

    bf16 = ml_dtypes.bfloat16
    xtf = np.ascontiguousarray(
        np.asarray(x, dtype=np.float32).transpose(0, 2, 1)
    ).astype(bf16)  # [B, D, T]

    # [d, o] -> [g, p, c, j]: d = c*128+p, o = g*128+j
    def swz(w):
        wt = np.asarray(w, np.float32).T.reshape(CH, P, CH, P)
        return np.ascontiguousarray(wt.transpose(2, 1, 0, 3)).astype(bf16)

    wqt = swz(w_q)
    wkt = swz(w_k)
    wvt = np.ascontiguousarray(np.asarray(w_v, np.float32).T).astype(bf16)
    wot = np.ascontiguousarray(np.asarray(w_o, np.float32).T).astype(bf16)
    bo = np.asarray(b_o, np.float32)
    kk = np.arange(P)[:, None]
    qq = np.arange(P)[None, :]
    mask = (kk <= qq).astype(np.float32).astype(bf16)
    ones_np = np.ones((P, DK), np.float32).astype(bf16)
    return {
        "xt": xtf,
        "wqt": wqt,
        "wkt": wkt,
        "wvt": wvt,
        "wot": wot,
        "bo": bo,
        "mask": mask,
        "ones": ones_np,
    }


def make_in_maps(host):
    return [
        {
            "xt": host["xt"][c * BL : (c + 1) * BL],
            "wqt": host["wqt"],
            "wkt": host["wkt"],
            "wvt": host["wvt"],
            "wot": host["wot"],
            "bo": host["bo"],
            "mask": host["mask"],
            "ones": host["ones"],
        }
        for c in range(NCORES)
    ]


LAST_RESULTS = None


def kernel(x, w_q, w_k, w_v, w_o, b_o):
    global LAST_RESULTS
    os.environ["BASS_NEVER_TRACE"] = "1"
    from concourse.bass_utils import run_bass_kernel_spmd

    host = host_inputs(x, w_q, w_k, w_v, w_o, b_o)
    nc = build(BL)
    core_ids = list(range(NCORES))
    res = run_bass_kernel_spmd(nc, make_in_maps(host), core_ids)
    LAST_RESULTS = res
    out = np.concatenate([res.results[c]["y"] for c in core_ids], axis=0)
    return out.astype(np.float32)
